# revision 37
# baseline (speedup 1.0000x reference)
"""Trainium2 Bass kernel for nn_ExactLookupMerger (vq_codebook) — v3.

Strategy (8 NeuronCores, data-parallel over batch B=8192, per the
sharding hint: replicate the ~128MB effective weights, shard B):
 - Host materializes W1_eff/W2_eff (codebook gather + frozen-mask merge)
   in bf16 and replicates them; each core takes a 1024-row batch slice.
 - Per core: Mbuild partial M = W2_sᵀ W1_sᵀ over its H-shard (+ v row),
   AllReduced (the ONLY collective) while GEMM1 runs.
 - Encode: h = c19(x @ W1 + b1) with full H per core (64 m-chunks,
   h resident in SBUF, 128KB/partition); c19 split across Act (tanh),
   DVE (affines), Pool (final add).
 - GEMM2: z = h @ W2 + b2, full-H contraction, local exact result.
 - Decode reassociated: recon = z @ M + (W1 db1 + db2), all local.
All GEMMs bf16 with f32 PSUM accumulation (rel err ~3.5e-3 measured).
"""
import sys

for _p in ("/opt/trn_rl_repo",):
    if _p not in sys.path:
        sys.path.insert(0, _p)

import numpy as np
import concourse.bass as bass
import concourse.tile as tile
from concourse import bacc, mybir
from concourse.bass_utils import run_bass_kernel_spmd

F32 = mybir.dt.float32
BF16 = mybir.dt.bfloat16
AF = mybir.ActivationFunctionType
OP = mybir.AluOpType

B, IN_D, H, OUT_D, CB = 8192, 2048, 8192, 2048, 256
NC = 8
BS = B // NC             # 1024 batch rows per core
KIN = IN_D // 128        # 16 contraction chunks for GEMM1
MH = H // 128            # 64 H chunks
MLO = OUT_D // 128       # 16 OUT chunks
MIN_ = IN_D // 128       # 16 IN chunks (decode output)
HSC = 8                  # H-shard chunks per core (1024/128) for Mbuild


def _build():
    nc = bacc.Bacc("TRN2", target_bir_lowering=False, debug=False, num_devices=NC)

    # ---- inputs (per core) ----
    xs_d = nc.dram_tensor("xs", [128, KIN * 1024], BF16, kind="ExternalInput")
    g1_d = nc.dram_tensor("g1", [128, MH, KIN * 128], BF16, kind="ExternalInput")
    g2_d = nc.dram_tensor("g2", [128, MLO, MH * 128], BF16, kind="ExternalInput")
    w1t_d = nc.dram_tensor("w1t", [128, HSC * IN_D], BF16, kind="ExternalInput")
    w1tf_d = nc.dram_tensor("w1tf", [128, MH, IN_D], BF16, kind="ExternalInput")
    w2m_d = nc.dram_tensor("w2m", [128, 2, MH * 128], BF16, kind="ExternalInput")
    crw_d = nc.dram_tensor("crw", [128, MH], F32, kind="ExternalInput")
    rrw_d = nc.dram_tensor("rrw", [128, MH], F32, kind="ExternalInput")
    b1_d = nc.dram_tensor("b1h", [128, MH], F32, kind="ExternalInput")
    db1_d = nc.dram_tensor("db1h", [128, HSC], BF16, kind="ExternalInput")
    b2_d = nc.dram_tensor("b2h", [128, MLO], F32, kind="ExternalInput")
    db2_d = nc.dram_tensor("db2h", [128, MIN_], F32, kind="ExternalInput")

    # ---- outputs (per core) ----
    z_out = nc.dram_tensor("z_out", [OUT_D, BS], F32, kind="ExternalOutput")
    recon_out = nc.dram_tensor("recon_out", [IN_D, BS], F32, kind="ExternalOutput")

    replica = [list(range(NC))]

    with tile.TileContext(nc) as tc:
        with (
            tc.tile_pool(name="params", bufs=1) as params,
            tc.tile_pool(name="dram", bufs=1, space="DRAM") as dram,
        ):
            m_stage = dram.tile([OUT_D + 1, IN_D], BF16, tag="mst")
            m_ar = dram.tile([OUT_D + 1, IN_D], BF16, tag="mar")
            z_dram = dram.tile([OUT_D, BS], BF16, tag="zd")

            # ---------- params / c19 precompute ----------
            # NOTE param layout: column m holds elements [m*128, (m+1)*128).
            with nc.named_scope("params"):
                craw = params.tile([128, 64], F32, tag="craw")
                rraw = params.tile([128, 64], F32, tag="rraw")
                b1s = params.tile([128, 64], F32, tag="b1s")
                b2s = params.tile([128, MLO], F32, tag="b2s")
                db2s = params.tile([128, MIN_], F32, tag="db2s")
                db1p = params.tile([128, HSC], BF16, tag="db1p")
                nc.sync.dma_start(craw[:], crw_d.ap())
                nc.sync.dma_start(rraw[:], rrw_d.ap())
                nc.sync.dma_start(b1s[:], b1_d.ap())
                nc.sync.dma_start(b2s[:], b2_d.ap())
                nc.sync.dma_start(db2s[:], db2_d.ap())
                nc.sync.dma_start(db1p[:], db1_d.ap())
                c_sb = params.tile([128, 64], F32, tag="c")
                rho = params.tile([128, 64], F32, tag="rho")
                inv_c = params.tile([128, 64], F32, tag="invc")
                s1 = params.tile([128, 64], F32, tag="s1")
                s2 = params.tile([128, 64], F32, tag="s2")
                b1c = params.tile([128, 64], F32, tag="b1c")
                tmp = params.tile([128, 64], F32, tag="tmp")
                exp_c = params.tile([128, 64], F32, tag="expc")
                nc.scalar.activation(exp_c[:], craw[:], AF.Exp)
                nc.scalar.activation(c_sb[:], exp_c[:], AF.Ln, bias=1.0)
                nc.scalar.activation(rho[:], rraw[:], AF.Sigmoid)
                nc.vector.reciprocal(inv_c[:], c_sb[:])
                nc.vector.tensor_scalar(tmp[:], rho[:], -1.0, 1.0, OP.mult, OP.add)
                nc.vector.tensor_tensor(s1[:], tmp[:], c_sb[:], OP.mult)
                nc.vector.tensor_tensor(s2[:], rho[:], b1s[:], OP.mult)
                nc.vector.tensor_tensor(b1c[:], b1s[:], inv_c[:], OP.mult)

            # ---------- long-lived SBUF: h (128KB/part), xs (32KB/part) ----------
            hctx = tc.tile_pool(name="hpool", bufs=1)
            hpool = hctx.__enter__()
            h_sb = [hpool.tile([128, 16 * 1024], BF16, tag=f"h{i}", name=f"h{i}")
                    for i in range(4)]
            xctx = tc.tile_pool(name="xspool", bufs=1)
            xsp = xctx.__enter__()
            xs = xsp.tile([128, KIN * 1024], BF16, tag="xs")

            # ---------- Mbuild: rows [256c, 256c+256) of M over FULL H ----------
            # Each core builds its own 256 M-rows (full contraction, streaming
            # the replicated W1T), then ONE AllGather (1MB/core vs the 8.4MB
            # AllReduce) places rank blocks directly into m_ar rows.
            with nc.named_scope("mbuild"), \
                    tc.tile_pool(name="w1tp", bufs=6) as w1tp, \
                    tc.tile_pool(name="w2cp", bufs=4) as w2cp, \
                    tc.tile_pool(name="mout", bufs=3) as mout, \
                    tc.tile_pool(name="pm", bufs=1, space="PSUM") as pm:
                pms = [pm.tile([128, 512], F32, tag=f"pm{j}", name=f"pm{j}")
                       for j in range(8)]
                for kc in range(MH):
                    w1c = w1tp.tile([128, IN_D], BF16, tag="w1c")
                    # alternate HWDGE queues: the 32MB W1T stream is the
                    # mbuild bottleneck on a single queue (~200GB/s)
                    eng = nc.sync if kc % 2 == 0 else nc.scalar
                    eng.dma_start(w1c[:], w1tf_d[:, kc, :])
                    w2c = w2cp.tile([128, 2 * 128], BF16, tag="w2c")
                    nc.sync.dma_start(
                        w2c.rearrange("p (m j) -> p m j", j=128),
                        w2m_d[:, :, kc * 128:(kc + 1) * 128],
                    )
                    if kc == 1:
                        # xs rides the DMA queue while Mbuild computes
                        nc.sync.dma_start(xs[:], xs_d.ap())
                    for mo in range(2):
                        for ih in range(4):
                            nc.tensor.matmul(
                                pms[mo * 4 + ih][:],
                                w2c[:, mo * 128:(mo + 1) * 128],
                                w1c[:, ih * 512:(ih + 1) * 512],
                                start=(kc == 0), stop=(kc == MH - 1),
                            )
                for mo in range(2):
                    for ih in range(4):
                        ms = mout.tile([128, 512], BF16, tag="ms")
                        nc.scalar.copy(ms[:], pms[mo * 4 + ih][:])
                        nc.sync.dma_start(
                            m_stage[mo * 128:(mo + 1) * 128,
                                    ih * 512:(ih + 1) * 512],
                            ms[:],
                        )
            with nc.named_scope("m_ag"):
                nc.gpsimd.collective_compute(
                    "AllGather", OP.bypass,
                    replica_groups=replica,
                    ins=[m_stage[0:2 * 128, :]],
                    outs=[m_ar[0:OUT_D, :]],
                )

            # ---------- vrow: v-partial = db1_s^T @ W1T_s over H-shard ----------
            with nc.named_scope("vrow"), \
                    tc.tile_pool(name="w1sp", bufs=1) as w1sp, \
                    tc.tile_pool(name="vout", bufs=2) as vout, \
                    tc.tile_pool(name="pv", bufs=2, space="PSUM") as pv:
                w1ts = []
                for kc in range(HSC):
                    w1tc = w1sp.tile([128, IN_D], BF16, tag=f"w1tc{kc}",
                                     name=f"w1tc{kc}")
                    nc.sync.dma_start(
                        w1tc[:], w1t_d[:, kc * IN_D:(kc + 1) * IN_D]
                    )
                    w1ts.append(w1tc)
                for iv in range(4):
                    pvt = pv.tile([1, 512], F32, tag="pv")
                    for kc in range(HSC):
                        nc.tensor.matmul(
                            pvt[:],
                            db1p[:, kc:kc + 1],
                            w1ts[kc][:, iv * 512:iv * 512 + 512],
                            start=(kc == 0), stop=(kc == HSC - 1),
                        )
                    vr = vout.tile([1, 512], BF16, tag="vr")
                    nc.scalar.copy(vr[:], pvt[:])
                    nc.sync.dma_start(
                        m_stage[OUT_D:OUT_D + 1, iv * 512:(iv + 1) * 512], vr[:]
                    )
            with nc.named_scope("v_ar"):
                nc.gpsimd.collective_compute(
                    "AllReduce", OP.add,
                    replica_groups=replica,
                    ins=[m_stage[OUT_D:OUT_D + 1, :]],
                    outs=[m_ar[OUT_D:OUT_D + 1, :]],
                )

            # ---------- GEMM1 + c19 -> h in SBUF ----------
            with nc.named_scope("gemm1"), \
                    tc.tile_pool(name="g1p", bufs=4) as g1p, \
                    tc.tile_pool(name="cstage", bufs=4) as cst, \
                    tc.tile_pool(name="p1", bufs=6, space="PSUM") as p1:
                for m in range(MH):
                    g1t = g1p.tile([128, KIN * 128], BF16, tag="g1t")
                    nc.sync.dma_start(g1t[:], g1_d[:, m, :])
                    for n in range(2):
                        ps = p1.tile([128, 512], F32, tag="ps1")
                        for k in range(KIN):
                            nc.tensor.matmul(
                                ps[:],
                                g1t[:, k * 128:(k + 1) * 128],
                                xs[:, k * 1024 + n * 512:k * 1024 + n * 512 + 512],
                                start=(k == 0), stop=(k == KIN - 1),
                            )
                        t_t = cst.tile([128, 512], BF16, tag="t")
                        nc.scalar.activation(
                            t_t[:], ps[:], AF.Tanh,
                            bias=b1c[:, m:m + 1], scale=inv_c[:, m:m + 1],
                        )
                        t2 = cst.tile([128, 512], BF16, tag="t2")
                        nc.vector.tensor_scalar(
                            t2[:], t_t[:], s1[:, m:m + 1], s2[:, m:m + 1],
                            OP.mult, OP.add,
                        )
                        ub = cst.tile([128, 512], BF16, tag="ub")
                        nc.vector.tensor_scalar(
                            ub[:], ps[:], rho[:, m:m + 1], None, OP.mult
                        )
                        nc.gpsimd.tensor_tensor(
                            h_sb[m // 16][:, (m % 16) * 1024 + n * 512:
                                          (m % 16) * 1024 + n * 512 + 512],
                            ub[:], t2[:], OP.add,
                        )
            xctx.__exit__(None, None, None)

            # first half of M prefetched on the right SBUF side during GEMM2
            mpactx = tc.tile_pool(name="mpa", bufs=1, side="right")
            mpa = mpactx.__enter__()
            m_sb = []
            for k in range(MLO // 2):
                mt = mpa.tile([128, IN_D], BF16, tag=f"m_{k}", name=f"m_{k}")
                nc.sync.dma_start(mt[:], m_ar[k * 128:(k + 1) * 128, :])
                m_sb.append(mt)

            # ---------- GEMM2: z = h @ W2 + b2 (full-H, local) ----------
            with nc.named_scope("gemm2"), \
                    tc.tile_pool(name="g2p", bufs=2) as g2p, \
                    tc.tile_pool(name="zstage", bufs=4) as zst, \
                    tc.tile_pool(name="p2", bufs=6, space="PSUM") as p2:
                for ml in range(MLO):
                    g2t = g2p.tile([128, MH * 128], BF16, tag="g2t")
                    nc.sync.dma_start(g2t[:], g2_d[:, ml, :])
                    for n in range(2):
                        ps = p2.tile([128, 512], F32, tag="ps2")
                        for k in range(MH):
                            nc.tensor.matmul(
                                ps[:],
                                g2t[:, k * 128:(k + 1) * 128],
                                h_sb[k // 16][:, (k % 16) * 1024 + n * 512:
                                              (k % 16) * 1024 + n * 512 + 512],
                                start=(k == 0), stop=(k == MH - 1),
                            )
                        zf = zst.tile([128, 512], F32, tag="zf")
                        nc.scalar.activation(
                            zf[:], ps[:], AF.Identity, bias=b2s[:, ml:ml + 1]
                        )
                        nc.sync.dma_start(
                            z_out[ml * 128:(ml + 1) * 128,
                                  n * 512:(n + 1) * 512],
                            zf[:],
                        )
                        zb = zst.tile([128, 512], BF16, tag="zb")
                        nc.vector.tensor_scalar(
                            zb[:], ps[:], b2s[:, ml:ml + 1], None, OP.add
                        )
                        nc.sync.dma_start(
                            z_dram[ml * 128:(ml + 1) * 128,
                                   n * 512:(n + 1) * 512],
                            zb[:],
                        )
            hctx.__exit__(None, None, None)

            # ---------- decode: recon = z @ M + (v + db2) ----------
            with nc.named_scope("decode"):
                with (
                    tc.tile_pool(name="mpool", bufs=1) as mpool,
                    tc.tile_pool(name="zsb", bufs=1) as zsbp,
                    tc.tile_pool(name="ro", bufs=4) as ro,
                    tc.tile_pool(name="p3", bufs=4, space="PSUM") as p3,
                ):
                    vdb = params.tile([128, MIN_], BF16, tag="vdb")
                    nc.sync.dma_start(
                        vdb[:],
                        m_ar[OUT_D:OUT_D + 1, :].rearrange(
                            "one (m p) -> (one p) m", p=128
                        ),
                    )
                    vd = params.tile([128, MIN_], F32, tag="vd")
                    nc.vector.tensor_copy(vd[:], vdb[:])
                    nc.vector.tensor_tensor(vd[:], vd[:], db2s[:], OP.add)
                    # interleave loads in first-use order: z_k is consumed at
                    # decode step k, m_k (k>=8) at step k; issue accordingly
                    z_sb = [None] * MLO
                    for k in range(MLO):
                        zt = zsbp.tile([128, BS], BF16, tag=f"z_{k}",
                                       name=f"z_{k}")
                        nc.sync.dma_start(zt[:], z_dram[k * 128:(k + 1) * 128, :])
                        z_sb[k] = zt
                        if k >= MLO // 2:
                            mt = mpool.tile([128, IN_D], BF16, tag=f"m_{k}",
                                            name=f"m_{k}")
                            nc.sync.dma_start(mt[:],
                                              m_ar[k * 128:(k + 1) * 128, :])
                            m_sb.append(mt)
                    for mi in range(MIN_):
                        for nh in range(2):
                            ps = p3.tile([128, 512], F32, tag="ps3")
                            for k in range(MLO):
                                nc.tensor.matmul(
                                    ps[:],
                                    m_sb[k][:, mi * 128:(mi + 1) * 128],
                                    z_sb[k][:, nh * 512:(nh + 1) * 512],
                                    start=(k == 0), stop=(k == MLO - 1),
                                )
                            rt = ro.tile([128, 512], F32, tag="ro")
                            nc.scalar.activation(
                                rt[:], ps[:], AF.Identity, bias=vd[:, mi:mi + 1]
                            )
                            nc.sync.dma_start(
                                recon_out[mi * 128:(mi + 1) * 128,
                                          nh * 512:(nh + 1) * 512],
                                rt[:],
                            )
            mpactx.__exit__(None, None, None)

    nc.compile()
    return nc


_CACHE = {}


def _get_nc():
    if "nc" not in _CACHE:
        _CACHE["nc"] = _build()
    return _CACHE["nc"]


def _prep_in_maps(inputs):
    bf16 = mybir.dt.np(BF16)
    x = np.asarray(inputs["x"], np.float32)
    cb1 = np.asarray(inputs["codebook_W1"], np.float32)
    cb2 = np.asarray(inputs["codebook_W2"], np.float32)
    W1m = np.asarray(inputs["W1_frozen_mask"])
    W2m = np.asarray(inputs["W2_frozen_mask"])
    W1 = np.where(W1m, cb1[np.asarray(inputs["W1_idx"], np.int64)],
                  np.asarray(inputs["W1_float"], np.float32)).astype(bf16)
    W2 = np.where(W2m, cb2[np.asarray(inputs["W2_idx"], np.int64)],
                  np.asarray(inputs["W2_float"], np.float32)).astype(bf16)
    b1 = np.asarray(inputs["b1"], np.float32)
    b2 = np.asarray(inputs["b2"], np.float32)
    db1 = np.asarray(inputs["db1"], np.float32)
    db2 = np.asarray(inputs["db2"], np.float32)
    craw = np.asarray(inputs["c19_c_raw"], np.float32)
    rraw = np.asarray(inputs["c19_rho_raw"], np.float32)

    # replicated weight layouts
    # g1[p, m, k*128+j] = W1[k*128+p, m*128+j]
    g1 = np.ascontiguousarray(
        W1.reshape(KIN, 128, MH, 128).transpose(1, 2, 0, 3)
        .reshape(128, MH, KIN * 128)
    )
    # g2[p, ml, k*128+j] = W2[k*128+p, ml*128+j]
    g2 = np.ascontiguousarray(
        W2.reshape(MH, 128, MLO, 128).transpose(1, 2, 0, 3)
        .reshape(128, MLO, MH * 128)
    )
    # w1tf[p, kc, i] = W1T[kc*128+p, i] = W1[i, kc*128+p] (replicated)
    w1tf = np.ascontiguousarray(
        W1.T.reshape(MH, 128, IN_D).transpose(1, 0, 2)
    )

    def p_cols(v, w):  # [w*128] -> [128, w] col-chunked
        return np.ascontiguousarray(v.reshape(w, 128).T)

    in_maps = []
    for c in range(NC):
        hs = slice(1024 * c, 1024 * (c + 1))
        bsl = slice(BS * c, BS * (c + 1))
        # w1t[p, kc*IN_D + i] = W1T[1024c + kc*128 + p, i] = W1[i, 1024c+kc*128+p]
        w1t = np.ascontiguousarray(
            W1[:, hs].T.reshape(HSC, 128, IN_D).transpose(1, 0, 2)
            .reshape(128, HSC * IN_D)
        )
        # w2m[p, mo, kc*128+j] = W2[kc*128+p, (2c+mo)*128+j] (M row-shard cols)
        w2m = np.ascontiguousarray(
            W2[:, 256 * c:256 * (c + 1)]
            .reshape(MH, 128, 2, 128).transpose(1, 2, 0, 3)
            .reshape(128, 2, MH * 128)
        )
        xsl = np.ascontiguousarray(
            x[bsl].T.reshape(KIN, 128, BS).transpose(1, 0, 2)
            .reshape(128, KIN * BS)
        ).astype(bf16)
        im = dict(
            xs=xsl, g1=g1, g2=g2, w1t=w1t, w1tf=w1tf, w2m=w2m,
            crw=p_cols(craw, 64), rrw=p_cols(rraw, 64), b1h=p_cols(b1, 64),
            db1h=np.ascontiguousarray(db1[hs].reshape(HSC, 128).T).astype(bf16),
            b2h=p_cols(b2, MLO), db2h=p_cols(db2, MIN_),
        )
        in_maps.append(im)
    return in_maps


def _assemble(results):
    reconT = np.concatenate([results[c]["recon_out"] for c in range(NC)], axis=1)
    zT = np.concatenate([results[c]["z_out"] for c in range(NC)], axis=1)
    recon = np.ascontiguousarray(reconT.T, dtype=np.float32)
    z = np.ascontiguousarray(zT.T, dtype=np.float32)
    return recon, z


def kernel(**inputs):
    nc = _get_nc()
    in_maps = _prep_in_maps(inputs)
    res = run_bass_kernel_spmd(nc, in_maps, core_ids=list(range(NC)))
    return _assemble(res.results)


# revision 38
# speedup vs baseline: 1.0120x; 1.0120x over previous
"""Trainium2 Bass kernel for nn_ExactLookupMerger (vq_codebook) — v3.

Strategy (8 NeuronCores, data-parallel over batch B=8192, per the
sharding hint: replicate the ~128MB effective weights, shard B):
 - Host materializes W1_eff/W2_eff (codebook gather + frozen-mask merge)
   in bf16 and replicates them; each core takes a 1024-row batch slice.
 - Per core: Mbuild partial M = W2_sᵀ W1_sᵀ over its H-shard (+ v row),
   AllReduced (the ONLY collective) while GEMM1 runs.
 - Encode: h = c19(x @ W1 + b1) with full H per core (64 m-chunks,
   h resident in SBUF, 128KB/partition); c19 split across Act (tanh),
   DVE (affines), Pool (final add).
 - GEMM2: z = h @ W2 + b2, full-H contraction, local exact result.
 - Decode reassociated: recon = z @ M + (W1 db1 + db2), all local.
All GEMMs bf16 with f32 PSUM accumulation (rel err ~3.5e-3 measured).
"""
import sys

for _p in ("/opt/trn_rl_repo",):
    if _p not in sys.path:
        sys.path.insert(0, _p)

import numpy as np
import concourse.bass as bass
import concourse.tile as tile
from concourse import bacc, mybir
from concourse.bass_utils import run_bass_kernel_spmd

F32 = mybir.dt.float32
BF16 = mybir.dt.bfloat16
AF = mybir.ActivationFunctionType
OP = mybir.AluOpType

B, IN_D, H, OUT_D, CB = 8192, 2048, 8192, 2048, 256
NC = 8
BS = B // NC             # 1024 batch rows per core
KIN = IN_D // 128        # 16 contraction chunks for GEMM1
MH = H // 128            # 64 H chunks
MLO = OUT_D // 128       # 16 OUT chunks
MIN_ = IN_D // 128       # 16 IN chunks (decode output)
HSC = 8                  # H-shard chunks per core (1024/128) for Mbuild


def _build():
    nc = bacc.Bacc("TRN2", target_bir_lowering=False, debug=False, num_devices=NC)

    # ---- inputs (per core) ----
    xs_d = nc.dram_tensor("xs", [128, KIN * 1024], BF16, kind="ExternalInput")
    g1_d = nc.dram_tensor("g1", [128, MH, KIN * 128], BF16, kind="ExternalInput")
    g2_d = nc.dram_tensor("g2", [128, MLO, MH * 128], BF16, kind="ExternalInput")
    w1t_d = nc.dram_tensor("w1t", [128, HSC * IN_D], BF16, kind="ExternalInput")
    w1tf_d = nc.dram_tensor("w1tf", [128, MH, IN_D], BF16, kind="ExternalInput")
    w2m_d = nc.dram_tensor("w2m", [128, 2, MH * 128], BF16, kind="ExternalInput")
    crw_d = nc.dram_tensor("crw", [128, MH], F32, kind="ExternalInput")
    rrw_d = nc.dram_tensor("rrw", [128, MH], F32, kind="ExternalInput")
    b1_d = nc.dram_tensor("b1h", [128, MH], F32, kind="ExternalInput")
    db1_d = nc.dram_tensor("db1h", [128, HSC], BF16, kind="ExternalInput")
    b2_d = nc.dram_tensor("b2h", [128, MLO], F32, kind="ExternalInput")
    db2_d = nc.dram_tensor("db2h", [128, MIN_], F32, kind="ExternalInput")

    # ---- outputs (per core) ----
    z_out = nc.dram_tensor("z_out", [OUT_D, BS], F32, kind="ExternalOutput")
    recon_out = nc.dram_tensor("recon_out", [IN_D, BS], F32, kind="ExternalOutput")

    replica = [list(range(NC))]

    with tile.TileContext(nc) as tc:
        with (
            tc.tile_pool(name="params", bufs=1) as params,
            tc.tile_pool(name="dram", bufs=1, space="DRAM") as dram,
        ):
            m_stage = dram.tile([OUT_D + 1, IN_D], BF16, tag="mst")
            m_ar = dram.tile([OUT_D + 1, IN_D], BF16, tag="mar")
            z_dram = dram.tile([OUT_D, BS], BF16, tag="zd")

            # ---------- params / c19 precompute ----------
            # NOTE param layout: column m holds elements [m*128, (m+1)*128).
            with nc.named_scope("params"):
                craw = params.tile([128, 64], F32, tag="craw")
                rraw = params.tile([128, 64], F32, tag="rraw")
                b1s = params.tile([128, 64], F32, tag="b1s")
                b2s = params.tile([128, MLO], F32, tag="b2s")
                db2s = params.tile([128, MIN_], F32, tag="db2s")
                db1p = params.tile([128, HSC], BF16, tag="db1p")
                nc.sync.dma_start(craw[:], crw_d.ap())
                nc.sync.dma_start(rraw[:], rrw_d.ap())
                nc.sync.dma_start(b1s[:], b1_d.ap())
                nc.sync.dma_start(b2s[:], b2_d.ap())
                nc.sync.dma_start(db2s[:], db2_d.ap())
                nc.sync.dma_start(db1p[:], db1_d.ap())
                c_sb = params.tile([128, 64], F32, tag="c")
                rho = params.tile([128, 64], F32, tag="rho")
                inv_c = params.tile([128, 64], F32, tag="invc")
                s1 = params.tile([128, 64], F32, tag="s1")
                s2 = params.tile([128, 64], F32, tag="s2")
                b1c = params.tile([128, 64], F32, tag="b1c")
                tmp = params.tile([128, 64], F32, tag="tmp")
                exp_c = params.tile([128, 64], F32, tag="expc")
                nc.scalar.activation(exp_c[:], craw[:], AF.Exp)
                nc.scalar.activation(c_sb[:], exp_c[:], AF.Ln, bias=1.0)
                nc.scalar.activation(rho[:], rraw[:], AF.Sigmoid)
                nc.vector.reciprocal(inv_c[:], c_sb[:])
                nc.vector.tensor_scalar(tmp[:], rho[:], -1.0, 1.0, OP.mult, OP.add)
                nc.vector.tensor_tensor(s1[:], tmp[:], c_sb[:], OP.mult)
                nc.vector.tensor_tensor(s2[:], rho[:], b1s[:], OP.mult)
                nc.vector.tensor_tensor(b1c[:], b1s[:], inv_c[:], OP.mult)

            # ---------- long-lived SBUF: h (128KB/part), xs (32KB/part) ----------
            hctx = tc.tile_pool(name="hpool", bufs=1)
            hpool = hctx.__enter__()
            h_sb = [hpool.tile([128, 16 * 1024], BF16, tag=f"h{i}", name=f"h{i}")
                    for i in range(4)]
            xctx = tc.tile_pool(name="xspool", bufs=1)
            xsp = xctx.__enter__()
            xs = xsp.tile([128, KIN * 1024], BF16, tag="xs")

            # ---------- vrow: v-partial = db1_s^T @ W1T_s over H-shard ----------
            with nc.named_scope("vrow"), \
                    tc.tile_pool(name="w1sp", bufs=1) as w1sp, \
                    tc.tile_pool(name="vout", bufs=2) as vout, \
                    tc.tile_pool(name="pv", bufs=2, space="PSUM") as pv:
                w1ts = []
                for kc in range(HSC):
                    w1tc = w1sp.tile([128, IN_D], BF16, tag=f"w1tc{kc}",
                                     name=f"w1tc{kc}")
                    nc.sync.dma_start(
                        w1tc[:], w1t_d[:, kc * IN_D:(kc + 1) * IN_D]
                    )
                    w1ts.append(w1tc)
                for iv in range(4):
                    pvt = pv.tile([1, 512], F32, tag="pv")
                    for kc in range(HSC):
                        nc.tensor.matmul(
                            pvt[:],
                            db1p[:, kc:kc + 1],
                            w1ts[kc][:, iv * 512:iv * 512 + 512],
                            start=(kc == 0), stop=(kc == HSC - 1),
                        )
                    vr = vout.tile([1, 512], BF16, tag="vr")
                    nc.scalar.copy(vr[:], pvt[:])
                    nc.sync.dma_start(
                        m_stage[OUT_D:OUT_D + 1, iv * 512:(iv + 1) * 512], vr[:]
                    )
            with nc.named_scope("v_ar"):
                nc.gpsimd.collective_compute(
                    "AllReduce", OP.add,
                    replica_groups=replica,
                    ins=[m_stage[OUT_D:OUT_D + 1, :]],
                    outs=[m_ar[OUT_D:OUT_D + 1, :]],
                )

            # ---------- Mbuild: rows [256c, 256c+256) of M over FULL H ----------
            # Each core builds its own 256 M-rows (full contraction, streaming
            # the replicated W1T), then ONE AllGather (1MB/core vs the 8.4MB
            # AllReduce) places rank blocks directly into m_ar rows.
            with nc.named_scope("mbuild"), \
                    tc.tile_pool(name="w1tp", bufs=6) as w1tp, \
                    tc.tile_pool(name="w2cp", bufs=4) as w2cp, \
                    tc.tile_pool(name="mout", bufs=3) as mout, \
                    tc.tile_pool(name="pm", bufs=1, space="PSUM") as pm:
                pms = [pm.tile([128, 512], F32, tag=f"pm{j}", name=f"pm{j}")
                       for j in range(8)]
                for kc in range(MH):
                    w1c = w1tp.tile([128, IN_D], BF16, tag="w1c")
                    # alternate HWDGE queues: the 32MB W1T stream is the
                    # mbuild bottleneck on a single queue (~200GB/s)
                    eng = nc.sync if kc % 2 == 0 else nc.scalar
                    eng.dma_start(w1c[:], w1tf_d[:, kc, :])
                    w2c = w2cp.tile([128, 2 * 128], BF16, tag="w2c")
                    nc.sync.dma_start(
                        w2c.rearrange("p (m j) -> p m j", j=128),
                        w2m_d[:, :, kc * 128:(kc + 1) * 128],
                    )
                    if kc == 40:
                        # xs rides the DMA queue late in Mbuild (issuing it
                        # early starves the w1c stream at the pipeline head)
                        nc.sync.dma_start(xs[:], xs_d.ap())
                    for mo in range(2):
                        for ih in range(4):
                            nc.tensor.matmul(
                                pms[mo * 4 + ih][:],
                                w2c[:, mo * 128:(mo + 1) * 128],
                                w1c[:, ih * 512:(ih + 1) * 512],
                                start=(kc == 0), stop=(kc == MH - 1),
                            )
                for mo in range(2):
                    for ih in range(4):
                        ms = mout.tile([128, 512], BF16, tag="ms")
                        nc.scalar.copy(ms[:], pms[mo * 4 + ih][:])
                        nc.sync.dma_start(
                            m_stage[mo * 128:(mo + 1) * 128,
                                    ih * 512:(ih + 1) * 512],
                            ms[:],
                        )
            with nc.named_scope("m_ag"):
                nc.gpsimd.collective_compute(
                    "AllGather", OP.bypass,
                    replica_groups=replica,
                    ins=[m_stage[0:2 * 128, :]],
                    outs=[m_ar[0:OUT_D, :]],
                )

            # ---------- GEMM1 + c19 -> h in SBUF ----------
            with nc.named_scope("gemm1"), \
                    tc.tile_pool(name="g1p", bufs=4) as g1p, \
                    tc.tile_pool(name="cstage", bufs=4) as cst, \
                    tc.tile_pool(name="p1", bufs=6, space="PSUM") as p1:
                for m in range(MH):
                    g1t = g1p.tile([128, KIN * 128], BF16, tag="g1t")
                    nc.sync.dma_start(g1t[:], g1_d[:, m, :])
                    for n in range(2):
                        ps = p1.tile([128, 512], F32, tag="ps1")
                        for k in range(KIN):
                            nc.tensor.matmul(
                                ps[:],
                                g1t[:, k * 128:(k + 1) * 128],
                                xs[:, k * 1024 + n * 512:k * 1024 + n * 512 + 512],
                                start=(k == 0), stop=(k == KIN - 1),
                            )
                        t_t = cst.tile([128, 512], BF16, tag="t")
                        nc.scalar.activation(
                            t_t[:], ps[:], AF.Tanh,
                            bias=b1c[:, m:m + 1], scale=inv_c[:, m:m + 1],
                        )
                        t2 = cst.tile([128, 512], BF16, tag="t2")
                        nc.vector.tensor_scalar(
                            t2[:], t_t[:], s1[:, m:m + 1], s2[:, m:m + 1],
                            OP.mult, OP.add,
                        )
                        ub = cst.tile([128, 512], BF16, tag="ub")
                        nc.vector.tensor_scalar(
                            ub[:], ps[:], rho[:, m:m + 1], None, OP.mult
                        )
                        nc.gpsimd.tensor_tensor(
                            h_sb[m // 16][:, (m % 16) * 1024 + n * 512:
                                          (m % 16) * 1024 + n * 512 + 512],
                            ub[:], t2[:], OP.add,
                        )
            xctx.__exit__(None, None, None)

            # first half of M prefetched on the right SBUF side during GEMM2
            mpactx = tc.tile_pool(name="mpa", bufs=1, side="right")
            mpa = mpactx.__enter__()
            m_sb = []
            for k in range(MLO // 2):
                mt = mpa.tile([128, IN_D], BF16, tag=f"m_{k}", name=f"m_{k}")
                nc.sync.dma_start(mt[:], m_ar[k * 128:(k + 1) * 128, :])
                m_sb.append(mt)

            # ---------- GEMM2: z = h @ W2 + b2 (full-H, local) ----------
            with nc.named_scope("gemm2"), \
                    tc.tile_pool(name="g2p", bufs=2) as g2p, \
                    tc.tile_pool(name="zstage", bufs=4) as zst, \
                    tc.tile_pool(name="p2", bufs=6, space="PSUM") as p2:
                for ml in range(MLO):
                    g2t = g2p.tile([128, MH * 128], BF16, tag="g2t")
                    nc.sync.dma_start(g2t[:], g2_d[:, ml, :])
                    for n in range(2):
                        ps = p2.tile([128, 512], F32, tag="ps2")
                        for k in range(MH):
                            nc.tensor.matmul(
                                ps[:],
                                g2t[:, k * 128:(k + 1) * 128],
                                h_sb[k // 16][:, (k % 16) * 1024 + n * 512:
                                              (k % 16) * 1024 + n * 512 + 512],
                                start=(k == 0), stop=(k == MH - 1),
                            )
                        zf = zst.tile([128, 512], F32, tag="zf")
                        nc.scalar.activation(
                            zf[:], ps[:], AF.Identity, bias=b2s[:, ml:ml + 1]
                        )
                        nc.sync.dma_start(
                            z_out[ml * 128:(ml + 1) * 128,
                                  n * 512:(n + 1) * 512],
                            zf[:],
                        )
                        zb = zst.tile([128, 512], BF16, tag="zb")
                        nc.vector.tensor_scalar(
                            zb[:], ps[:], b2s[:, ml:ml + 1], None, OP.add
                        )
                        nc.sync.dma_start(
                            z_dram[ml * 128:(ml + 1) * 128,
                                   n * 512:(n + 1) * 512],
                            zb[:],
                        )
            hctx.__exit__(None, None, None)

            # ---------- decode: recon = z @ M + (v + db2) ----------
            with nc.named_scope("decode"):
                with (
                    tc.tile_pool(name="mpool", bufs=1) as mpool,
                    tc.tile_pool(name="zsb", bufs=1) as zsbp,
                    tc.tile_pool(name="ro", bufs=4) as ro,
                    tc.tile_pool(name="p3", bufs=4, space="PSUM") as p3,
                ):
                    vdb = params.tile([128, MIN_], BF16, tag="vdb")
                    nc.sync.dma_start(
                        vdb[:],
                        m_ar[OUT_D:OUT_D + 1, :].rearrange(
                            "one (m p) -> (one p) m", p=128
                        ),
                    )
                    vd = params.tile([128, MIN_], F32, tag="vd")
                    nc.vector.tensor_copy(vd[:], vdb[:])
                    nc.vector.tensor_tensor(vd[:], vd[:], db2s[:], OP.add)
                    # interleave loads in first-use order: z_k is consumed at
                    # decode step k, m_k (k>=8) at step k; issue accordingly
                    z_sb = [None] * MLO
                    for k in range(MLO):
                        zt = zsbp.tile([128, BS], BF16, tag=f"z_{k}",
                                       name=f"z_{k}")
                        nc.sync.dma_start(zt[:], z_dram[k * 128:(k + 1) * 128, :])
                        z_sb[k] = zt
                        if k >= MLO // 2:
                            mt = mpool.tile([128, IN_D], BF16, tag=f"m_{k}",
                                            name=f"m_{k}")
                            nc.sync.dma_start(mt[:],
                                              m_ar[k * 128:(k + 1) * 128, :])
                            m_sb.append(mt)
                    for mi in range(MIN_):
                        for nh in range(2):
                            ps = p3.tile([128, 512], F32, tag="ps3")
                            for k in range(MLO):
                                nc.tensor.matmul(
                                    ps[:],
                                    m_sb[k][:, mi * 128:(mi + 1) * 128],
                                    z_sb[k][:, nh * 512:(nh + 1) * 512],
                                    start=(k == 0), stop=(k == MLO - 1),
                                )
                            rt = ro.tile([128, 512], F32, tag="ro")
                            nc.scalar.activation(
                                rt[:], ps[:], AF.Identity, bias=vd[:, mi:mi + 1]
                            )
                            nc.sync.dma_start(
                                recon_out[mi * 128:(mi + 1) * 128,
                                          nh * 512:(nh + 1) * 512],
                                rt[:],
                            )
            mpactx.__exit__(None, None, None)

    nc.compile()
    return nc


_CACHE = {}


def _get_nc():
    if "nc" not in _CACHE:
        _CACHE["nc"] = _build()
    return _CACHE["nc"]


def _prep_in_maps(inputs):
    bf16 = mybir.dt.np(BF16)
    x = np.asarray(inputs["x"], np.float32)
    cb1 = np.asarray(inputs["codebook_W1"], np.float32)
    cb2 = np.asarray(inputs["codebook_W2"], np.float32)
    W1m = np.asarray(inputs["W1_frozen_mask"])
    W2m = np.asarray(inputs["W2_frozen_mask"])
    W1 = np.where(W1m, cb1[np.asarray(inputs["W1_idx"], np.int64)],
                  np.asarray(inputs["W1_float"], np.float32)).astype(bf16)
    W2 = np.where(W2m, cb2[np.asarray(inputs["W2_idx"], np.int64)],
                  np.asarray(inputs["W2_float"], np.float32)).astype(bf16)
    b1 = np.asarray(inputs["b1"], np.float32)
    b2 = np.asarray(inputs["b2"], np.float32)
    db1 = np.asarray(inputs["db1"], np.float32)
    db2 = np.asarray(inputs["db2"], np.float32)
    craw = np.asarray(inputs["c19_c_raw"], np.float32)
    rraw = np.asarray(inputs["c19_rho_raw"], np.float32)

    # replicated weight layouts
    # g1[p, m, k*128+j] = W1[k*128+p, m*128+j]
    g1 = np.ascontiguousarray(
        W1.reshape(KIN, 128, MH, 128).transpose(1, 2, 0, 3)
        .reshape(128, MH, KIN * 128)
    )
    # g2[p, ml, k*128+j] = W2[k*128+p, ml*128+j]
    g2 = np.ascontiguousarray(
        W2.reshape(MH, 128, MLO, 128).transpose(1, 2, 0, 3)
        .reshape(128, MLO, MH * 128)
    )
    # w1tf[p, kc, i] = W1T[kc*128+p, i] = W1[i, kc*128+p] (replicated)
    w1tf = np.ascontiguousarray(
        W1.T.reshape(MH, 128, IN_D).transpose(1, 0, 2)
    )

    def p_cols(v, w):  # [w*128] -> [128, w] col-chunked
        return np.ascontiguousarray(v.reshape(w, 128).T)

    in_maps = []
    for c in range(NC):
        hs = slice(1024 * c, 1024 * (c + 1))
        bsl = slice(BS * c, BS * (c + 1))
        # w1t[p, kc*IN_D + i] = W1T[1024c + kc*128 + p, i] = W1[i, 1024c+kc*128+p]
        w1t = np.ascontiguousarray(
            W1[:, hs].T.reshape(HSC, 128, IN_D).transpose(1, 0, 2)
            .reshape(128, HSC * IN_D)
        )
        # w2m[p, mo, kc*128+j] = W2[kc*128+p, (2c+mo)*128+j] (M row-shard cols)
        w2m = np.ascontiguousarray(
            W2[:, 256 * c:256 * (c + 1)]
            .reshape(MH, 128, 2, 128).transpose(1, 2, 0, 3)
            .reshape(128, 2, MH * 128)
        )
        xsl = np.ascontiguousarray(
            x[bsl].T.reshape(KIN, 128, BS).transpose(1, 0, 2)
            .reshape(128, KIN * BS)
        ).astype(bf16)
        im = dict(
            xs=xsl, g1=g1, g2=g2, w1t=w1t, w1tf=w1tf, w2m=w2m,
            crw=p_cols(craw, 64), rrw=p_cols(rraw, 64), b1h=p_cols(b1, 64),
            db1h=np.ascontiguousarray(db1[hs].reshape(HSC, 128).T).astype(bf16),
            b2h=p_cols(b2, MLO), db2h=p_cols(db2, MIN_),
        )
        in_maps.append(im)
    return in_maps


def _assemble(results):
    reconT = np.concatenate([results[c]["recon_out"] for c in range(NC)], axis=1)
    zT = np.concatenate([results[c]["z_out"] for c in range(NC)], axis=1)
    recon = np.ascontiguousarray(reconT.T, dtype=np.float32)
    z = np.ascontiguousarray(zT.T, dtype=np.float32)
    return recon, z


def kernel(**inputs):
    nc = _get_nc()
    in_maps = _prep_in_maps(inputs)
    res = run_bass_kernel_spmd(nc, in_maps, core_ids=list(range(NC)))
    return _assemble(res.results)
